# revision 7
# baseline (speedup 1.0000x reference)
"""Distributed Trainium2 kernel for the ADD rotation loss.

Math: the reference computes mean_{b,n} || point[b,n] @ (R_pred[b] - R_gt[b]) ||
with R_pred/R_gt rotation matrices. Because both are rotations,

    || p @ (Rp - Rg) || = 2 * | p x qv |,

where qv is the vector part of the relative quaternion q_pred * conj(q_gt).
The pred-side euler angles enter only through cos/sin, which reduce to pure
arithmetic (no arcsin/arctan2 needed); only the gt side needs real sin().

Per core (data-parallel over batch): stream the point shard once from HBM,
form the cross product with per-batch-row constants via TensorEngine matmuls
with *diagonal* stationary matrices (float32r -> full PE rate, no cast pass),
square on ACT/DVE, sqrt + per-row accumulate on ACT, and emit per-row partial
sums. The final tiny reduction (8 cores x 128 x 8 values) happens on host.
"""

import sys

for _p in ("/opt/trn_rl_repo", "/root/.axon_site/_ro/trn_rl_repo"):
    if _p not in sys.path:
        sys.path.append(_p)

import math

import numpy as np

import concourse.bacc as bacc
import concourse.tile as tile
from concourse import mybir
from concourse.bass_utils import run_bass_kernel_spmd

NCORES = 8
B = 8192
N = 1024
BSH = B // NCORES          # batch rows per core
G = BSH // 128             # b-groups of 128 rows per core
ROW = 3 * N                # floats per point row
HALF = 3 * (N // 2)        # floats per half row
F32 = mybir.dt.float32
F32R = mybir.dt.float32r
BF16 = mybir.dt.bfloat16
OP = mybir.AluOpType
AF = mybir.ActivationFunctionType

_CACHE = {}


def build_bass():
    nc = bacc.Bacc("TRN2", target_bir_lowering=False, debug=False,
                   num_devices=NCORES)
    pred = nc.declare_dram_parameter("pred", [BSH, 4], F32, isOutput=False)
    mode = nc.declare_dram_parameter("mode", [BSH, 1], F32, isOutput=False)
    gt = nc.declare_dram_parameter("gt", [BSH, 3], F32, isOutput=False)
    point = nc.declare_dram_parameter("point", [BSH, ROW], F32, isOutput=False)
    out = nc.declare_dram_parameter("out", [128, G], F32, isOutput=True)

    with tile.TileContext(nc) as tc:
        with (
            tc.tile_pool(name="coef", bufs=1) as cp,
            tc.tile_pool(name="data", bufs=3) as dp,
            tc.tile_pool(name="diag", bufs=2) as gp,
            tc.tile_pool(name="sq", bufs=2) as qp,
            tc.tile_pool(name="psum", bufs=1, space="PSUM") as pp,
        ):
            uid = [0]

            def ctile(shape, dtype=F32):
                uid[0] += 1
                return cp.tile(shape, dtype, name=f"c{uid[0]}",
                               tag=f"c{uid[0]}")

            def tt(in0, in1, op, shape=None, out=None):
                """out = in0 op in1 (DVE); returns the written AP."""
                if out is None:
                    out = ctile(shape if shape is not None else [128, G])
                nc.vector.tensor_tensor(out=out, in0=in0, in1=in1, op=op)
                return out

            def ts(in0, s1, s2, op0, op1=None, out=None, shape=None):
                if out is None:
                    out = ctile(shape if shape is not None else [128, G])
                if op1 is None:
                    nc.vector.tensor_scalar(out=out, in0=in0, scalar1=s1,
                                            scalar2=None, op0=op0)
                else:
                    nc.vector.tensor_scalar(out=out, in0=in0, scalar1=s1,
                                            scalar2=s2, op0=op0, op1=op1)
                return out

            def recip(in_, shape=None):
                o = ctile(shape if shape is not None else [128, G])
                nc.vector.reciprocal(out=o, in_=in_)
                return o

            _consts = {}

            def constcol(val):
                if val not in _consts:
                    uid[0] += 1
                    t = cp.tile([128, 1], F32, name=f"k{uid[0]}",
                                tag=f"k{uid[0]}")
                    nc.vector.memset(t[:, :], val)
                    _consts[val] = t
                return _consts[val]

            def act(in_, func, scale=1.0, bias=0.0, out=None, shape=None):
                if out is None:
                    out = ctile(shape if shape is not None else [128, G])
                if isinstance(bias, float) and bias != 0.0:
                    bias = constcol(bias)[:, :]
                nc.scalar.activation(out=out, in_=in_, func=func,
                                     scale=scale, bias=bias)
                return out

            # ---- coefficient inputs, transposed so row b=128g+p is at [p,g] ----
            cgt = ctile([128, G, 3])
            nc.sync.dma_start(out=cgt[:, :, :],
                              in_=gt[:, :].rearrange("(g p) c -> p g c", p=128))
            cpred = ctile([128, G, 4])
            nc.sync.dma_start(out=cpred[:, :, :],
                              in_=pred[:, :].rearrange("(g p) c -> p g c", p=128))
            cmode = ctile([128, G, 1])
            nc.sync.dma_start(out=cmode[:, :, :],
                              in_=mode[:, :].rearrange("(g p) c -> p g c", p=128))

            # gt half-angle cos/sin first: the two Sin ops run before anything
            # needing the sqrt table set, so ACT loads each table set once.
            chg = act(cgt[:, :, :], AF.Sin, scale=0.5, bias=math.pi / 2,
                      shape=[128, G, 3])
            shg = act(cgt[:, :, :], AF.Sin, scale=0.5, shape=[128, G, 3])

            # ---- pred side: cos/sin of euler angles, arithmetic only ----
            m1, m2 = cpred[:, :, 0], cpred[:, :, 1]
            m3, m4 = cpred[:, :, 2], cpred[:, :, 3]
            msq = tt(cpred[:, :, :], cpred[:, :, :], OP.mult, shape=[128, G, 4])
            m1sq, m2sq = msq[:, :, 0], msq[:, :, 1]
            m3sq, m4sq = msq[:, :, 2], msq[:, :, 3]
            rxy = tt(m1sq, m2sq, OP.add)
            r = tt(rxy, m3sq, OP.add)
            rinv = recip(r)

            cc = ctile([128, G, 3])   # cos(e1..e3)
            ss = ctile([128, G, 3])   # sin(e1..e3)

            # e2: sin = sgn*sqrt(m3^2/r), cos = sqrt((m1^2+m2^2)/r)
            s2sq = tt(m3sq, rinv, OP.mult)
            c2sq = tt(rxy, rinv, OP.mult)
            s2a = act(s2sq, AF.Sqrt)
            act(c2sq, AF.Sqrt, out=cc[:, :, 1])
            sgn = act(cmode[:, :, 0], AF.Sign, bias=-0.5)
            tt(s2a, sgn, OP.mult, out=ss[:, :, 1])

            # e3: w = m3/(sin(e2)+1e-9); cos/sin = (w, m4)/hyp(w, m4)
            s2e = ts(ss[:, :, 1], 1e-9, None, OP.add)
            s2ei = recip(s2e)
            w = tt(m3, s2ei, OP.mult)
            wsq = tt(w, w, OP.mult)
            h3sq = tt(wsq, m4sq, OP.add)
            h3si = recip(h3sq)
            h3i = act(h3si, AF.Sqrt)
            tt(w, h3i, OP.mult, out=cc[:, :, 2])
            tt(m4, h3i, OP.mult, out=ss[:, :, 2])

            # e1: cos/sin = sign(cos2*cos3) * (m1, m2)/hyp(m1, m2)
            tmp = tt(cc[:, :, 1], cc[:, :, 2], OP.mult)
            sgnt = act(tmp, AF.Sign)
            rxyi = recip(rxy)
            hyi = act(rxyi, AF.Sqrt)
            c1a = tt(m1, hyi, OP.mult)
            s1a = tt(m2, hyi, OP.mult)
            tt(c1a, sgnt, OP.mult, out=cc[:, :, 0])
            tt(s1a, sgnt, OP.mult, out=ss[:, :, 0])

            # clamp cosines into [-1, 1] so the half-angle sqrts stay real
            ts(cc, 1.0, -1.0, OP.min, OP.max, out=cc, shape=None)

            # pred half-angle: ch = sqrt((1+c)/2), sh = sign(s)*sqrt((1-c)/2)
            chp = act(cc, AF.Sqrt, scale=0.5, bias=0.5, shape=[128, G, 3])
            shab = act(cc, AF.Sqrt, scale=-0.5, bias=0.5, shape=[128, G, 3])
            ssgn = act(ss, AF.Sign, shape=[128, G, 3])
            shp = tt(shab, ssgn, OP.mult, shape=[128, G, 3])

            # ---- quaternions: q = qx(e1) * qy(e2) * qz(e3) ----
            def quat_xyz(ch, sh):
                c1h, s1h = ch[:, :, 0], sh[:, :, 0]
                c2h, s2h = ch[:, :, 1], sh[:, :, 1]
                c3h, s3h = ch[:, :, 2], sh[:, :, 2]
                w12 = tt(c1h, c2h, OP.mult)
                x12 = tt(s1h, c2h, OP.mult)
                y12 = tt(c1h, s2h, OP.mult)
                z12 = tt(s1h, s2h, OP.mult)
                wq = tt(tt(w12, c3h, OP.mult), tt(z12, s3h, OP.mult), OP.subtract)
                xq = tt(tt(x12, c3h, OP.mult), tt(y12, s3h, OP.mult), OP.add)
                yq = tt(tt(y12, c3h, OP.mult), tt(x12, s3h, OP.mult), OP.subtract)
                zq = tt(tt(w12, s3h, OP.mult), tt(z12, c3h, OP.mult), OP.add)
                return wq, xq, yq, zq

            wp, xp, yp, zp = quat_xyz(chp, shp)
            wg, xg, yg, zg = quat_xyz(chg, shg)

            # qv = vec(q_pred * conj(q_gt))
            def sub4(a, b, c, d):
                # a - b - c + d
                return tt(tt(a, b, OP.subtract), tt(c, d, OP.subtract),
                          OP.subtract)

            qvx = sub4(tt(xp, wg, OP.mult), tt(wp, xg, OP.mult),
                       tt(yp, zg, OP.mult), tt(zp, yg, OP.mult))
            qvy = sub4(tt(yp, wg, OP.mult), tt(wp, yg, OP.mult),
                       tt(zp, xg, OP.mult), tt(xp, zg, OP.mult))
            qvz = sub4(tt(zp, wg, OP.mult), tt(wp, zg, OP.mult),
                       tt(xp, yg, OP.mult), tt(yp, xg, OP.mult))

            # ---- +/- identity pair for building diag stationaries ----
            pm1 = cp.tile([128, 256], F32, name="pm1", tag="pm1")
            nc.vector.memset(pm1[:, 0:128], 1.0)
            nc.vector.memset(pm1[:, 128:256], -1.0)
            ipm = cp.tile([128, 256], F32, name="ipm", tag="ipm")
            nc.gpsimd.affine_select(out=ipm[:], in_=pm1[:],
                                    pattern=[[0, 2], [-1, 128]],
                                    compare_op=OP.is_equal, fill=0.0,
                                    base=0, channel_multiplier=1)

            acc = cp.tile([128, G], F32, name="acc", tag="acc")

            # ---- main loop over b-groups ----
            for g in range(G):
                T = dp.tile([128, ROW], F32R, name="T", tag="T")
                nc.sync.dma_start(out=T[:, :],
                                  in_=point[g * 128:(g + 1) * 128, :].bitcast(F32R))

                # [diag(qv_c) | diag(-qv_c)] in one tensor_scalar each
                dx = gp.tile([128, 256], F32R, name="dx", tag="dx")
                dy = gp.tile([128, 256], F32R, name="dy", tag="dy")
                dz = gp.tile([128, 256], F32R, name="dz", tag="dz")
                for d, q in ((dx, qvx), (dy, qvy), (dz, qvz)):
                    nc.vector.tensor_scalar(out=d[:], in0=ipm[:],
                                            scalar1=q[:, g:g + 1], scalar2=None,
                                            op0=OP.mult)

                ps_cx = pp.tile([128, N], F32, name="ps_cx", tag="ps_cx")
                ps_cy = pp.tile([128, N], F32, name="ps_cy", tag="ps_cy")
                ps_cz = pp.tile([128, N], F32, name="ps_cz", tag="ps_cz")
                for h in range(2):
                    o = h * HALF
                    xv = T[:, o + 0:o + HALF:3]
                    yv = T[:, o + 1:o + HALF:3]
                    zv = T[:, o + 2:o + HALF:3]
                    col = slice(h * (N // 2), (h + 1) * (N // 2))
                    # cx = y*qvz - z*qvy ; cy = z*qvx - x*qvz ; cz = x*qvy - y*qvx
                    for ps, (w0, v0), (w1, v1) in (
                        (ps_cx, (dz, yv), (dy, zv)),
                        (ps_cy, (dx, zv), (dz, xv)),
                        (ps_cz, (dy, xv), (dx, yv)),
                    ):
                        nc.tensor.matmul(out=ps[:, col],
                                         lhsT=w0[:, 0:128],
                                         rhs=v0, start=True, stop=False)
                        nc.tensor.matmul(out=ps[:, col],
                                         lhsT=w1[:, 128:256],
                                         rhs=v1, start=False, stop=True)

                sqx = qp.tile([128, N], BF16, name="sqx", tag="sqx")
                sqy = qp.tile([128, N], BF16, name="sqy", tag="sqy")
                sqz = qp.tile([128, N], BF16, name="sqz", tag="sqz")
                nc.scalar.activation(out=sqx[:], in_=ps_cx[:], func=AF.Square)
                nc.scalar.activation(out=sqy[:], in_=ps_cy[:], func=AF.Square)
                nc.scalar.activation(out=sqz[:], in_=ps_cz[:], func=AF.Square)

                s01 = qp.tile([128, N], BF16, name="s01", tag="s01")
                stot = qp.tile([128, N], BF16, name="stot", tag="stot")
                nc.vector.tensor_tensor(out=s01[:], in0=sqx[:], in1=sqy[:],
                                        op=OP.add)
                nc.vector.tensor_tensor(out=stot[:], in0=s01[:], in1=sqz[:],
                                        op=OP.add)

                dists = qp.tile([128, N], BF16, name="dists", tag="dists")
                nc.scalar.activation(out=dists[:], in_=stot[:], func=AF.Sqrt,
                                     scale=4.0, accum_out=acc[:, g:g + 1])

            nc.sync.dma_start(out=out[:, :], in_=acc[:, :])

    nc.finalize()
    return nc


def _get_nc():
    if "nc" not in _CACHE:
        _CACHE["nc"] = build_bass()
    return _CACHE["nc"]


def kernel(pred, mode, gt, point, **run_kwargs):
    nc = _get_nc()
    in_maps = []
    for c in range(NCORES):
        sl = slice(c * BSH, (c + 1) * BSH)
        in_maps.append({
            "pred": np.ascontiguousarray(pred[sl], dtype=np.float32),
            "mode": np.ascontiguousarray(mode[sl], dtype=np.float32).reshape(BSH, 1),
            "gt": np.ascontiguousarray(gt[sl], dtype=np.float32),
            "point": np.ascontiguousarray(point[sl], dtype=np.float32).reshape(BSH, ROW),
        })
    res = run_bass_kernel_spmd(nc, in_maps, core_ids=list(range(NCORES)),
                               **run_kwargs)
    total = sum(float(r["out"].astype(np.float64).sum()) for r in res.results)
    result = np.float32(total / (B * N))
    if run_kwargs:
        return result, res
    return result
